# revision 24
# baseline (speedup 1.0000x reference)
"""Multi-head self-attention (causal) Trainium2 kernel, 8-way sharded.

Sharding: core c handles batch b = c//4 and head group g = c%4 (4 of 16
heads). Each core computes q/k/v projections for its head slice, causal
softmax attention, and a partial o_proj ([2048, 1024] bf16); the host
sums the 4 partials per batch in f32.

Layouts (per core):
  xT    [1024, 2048]  x[b].T            (d_model on partitions)
  wqT   [1024,  256]  Wq[g*256:(g+1)*256, :].T      (same for wk/wv)
  woT   [ 256, 1024]  Wo[:, g*256:(g+1)*256].T
  utri  [ 128,  640]  -1e9 where p > cc-128 (causal bias, shifted views)
  ident [ 128,  128]  identity (stationary for the causal-bias matmul)

All DRAM params are declared f32r (same bits as f32) and DMA'd straight
into f32r SBUF tiles — no rounding casts. q/k projections run ko-outer
(4 query-chunk PSUM groups live per weight half) so matmuls start as
soon as the first x chunk lands. kT/qT are stored head-major stacked
two heads per partition column; S matmuls contract over K=64 partition
slices so no zero padding is needed.

Attention per (query-pair pr, head): key tiles stream, each kt producing
one [128, <=1024] PSUM group. Diagonal sub-blocks get the causal mask as
an extra accumulated matmul (ident.T @ utri-slice = -1e9 above the
diagonal) so the single wide exp (ACT) yields exact zeros there and no
post-exp mask is needed. Diagonal chunk widths are kept >= 256 (f32r
matmuls below 256 moving columns run at 1/4 rate). O matmuls are issued
one kt-group late; V carries an appended ones column so PSUM partition
64 accumulates the softmax sums; normalization is a custom-DVE
reciprocal_approx_fast + GpSimd partition_broadcast + DVE multiply.
"""

import numpy as np

import concourse.bass as bass
import concourse.mybir as mybir
import concourse.tile as tile
from concourse import bacc
from concourse.bass_utils import run_bass_kernel_spmd

P = 128
S = 2048  # sequence length
DM = 1024  # d_model
HD = 64  # head dim
NH_CORE = 4  # heads per core
HSL = NH_CORE * HD  # head slice width = 256
QC = 512  # query chunk
N_QC = S // QC  # 4
N_KT = S // P  # 16 key tiles
KO = DM // P  # 8 k-tiles over d_model

f32 = mybir.dt.float32
f32r = mybir.dt.float32r
bf16 = mybir.dt.bfloat16

_CACHED = {}
DEBUG = False


def build_program():
    nc = bacc.Bacc("TRN2", target_bir_lowering=False, debug=False)
    # all inputs host-prearranged into SBUF tile layouts so every DMA line
    # is one long contiguous read per partition (no strided descriptors)
    xS = nc.declare_dram_parameter("xS", [N_QC, P, KO, QC], f32r, isOutput=False)
    wqT = nc.declare_dram_parameter("wqT", [P, KO, HSL], f32r, isOutput=False)
    wkT = nc.declare_dram_parameter("wkT", [P, KO, HSL], f32r, isOutput=False)
    wvT = nc.declare_dram_parameter("wvT", [P, KO, HSL], f32r, isOutput=False)
    woT = nc.declare_dram_parameter("woT", [P, 2, DM], f32r, isOutput=False)
    utri = nc.declare_dram_parameter("utri", [P, 5 * P], f32r, isOutput=False)
    ident = nc.declare_dram_parameter("ident", [P, P], f32r, isOutput=False)
    out = nc.declare_dram_parameter("out", [S, DM], bf16, isOutput=True)
    if DEBUG:
        dbg = {
            "qTr_d": nc.declare_dram_parameter(
                "qTr_d", [P, 2, S], f32r, isOutput=True
            ),
            "kTr_d": nc.declare_dram_parameter(
                "kTr_d", [P, 2, S], f32r, isOutput=True
            ),
            "vr_d": nc.declare_dram_parameter(
                "vr_d", [P, N_KT, NH_CORE, HD + 1], f32r, isOutput=True
            ),
            "aTr_d": nc.declare_dram_parameter(
                "aTr_d", [P, 2, S], f32r, isOutput=True
            ),
            "er_d": nc.declare_dram_parameter(
                "er_d", [P, 2, 2 * QC], f32r, isOutput=True
            ),
            "ot_d": nc.declare_dram_parameter(
                "ot_d", [HD + 1, QC], f32, isOutput=True
            ),
            "recip_d": nc.declare_dram_parameter(
                "recip_d", [1, QC], f32, isOutput=True
            ),
            "bcast_d": nc.declare_dram_parameter(
                "bcast_d", [HD, QC], f32, isOutput=True
            ),
        }

    with tile.TileContext(nc) as tc:
        with (
            tc.tile_pool(name="persist", bufs=1) as persist,
            tc.tile_pool(name="small", bufs=2) as small,
        ):
            # ---- persistent tiles
            qTr = persist.tile([P, NH_CORE, S], f32r, tag="qTr")
            kTr = persist.tile([P, 2, S], f32r, tag="kTr")
            vr = persist.tile([P, N_KT, NH_CORE, HD + 1], f32r, tag="vr")
            woTr = persist.tile([P, 2, DM], f32r, tag="woTr")
            aTr = persist.tile([P, 2, S], f32r, tag="aTr")
            utri_sb = persist.tile([P, 5 * P], f32r, tag="utri")
            ident_sb = persist.tile([P, P], f32r, tag="ident")

            # ---- phase 0+1: load x/weights, projections.
            # x streams as 4 seq-slabs of 512 cols (all d_model rows); each
            # slab feeds all four q/k projection halves plus its 4 v seq
            # tiles, so PE consumption (~12us) outpaces slab DMA (~6us) and
            # the tensor engine never idles (keeps the HAM clock at 2.4GHz).
            with tc.tile_pool(name="xw", bufs=1) as xw:
                xTr = xw.tile([P, KO, S], f32r, tag="xTr")
                wts = {}
                for name, dram in (("q", wqT), ("k", wkT), ("v", wvT)):
                    wts[name] = xw.tile(
                        [P, KO, HSL], f32r, tag=f"w{name}r", name=f"w{name}r"
                    )
                nc.sync.dma_start(wts["q"][:], wqT[:])
                nc.sync.dma_start(wts["k"][:], wkT[:])
                slab_qs = [nc.scalar, nc.gpsimd]
                for sl in range(N_QC):
                    ssl = slice(sl * QC, (sl + 1) * QC)
                    slab_qs[sl % 2].dma_start(xTr[:, :, ssl], xS[sl])
                nc.sync.dma_start(wts["v"][:], wvT[:])
                nc.sync.dma_start(woTr[:], woT[:])
                nc.sync.dma_start(utri_sb[:], utri[:])
                nc.sync.dma_start(ident_sb[:], ident[:])
                nc.vector.memset(
                    vr[:, :, :, HD]
                    .rearrange("p a b -> p (a b)")
                    .bitcast(mybir.dt.float32),
                    1.0,
                )
                # zero the pad halves of qTr: even heads live on partitions
                # 0:64, odd heads on 64:128 (the other half multiplies the
                # co-resident head's k rows, so it must be zero)
                zeros_f = persist.tile([P, 1], f32, tag="zeros")
                nc.vector.memset(zeros_f[:], 0.0)
                nc.vector.tensor_copy(
                    qTr[HD:P, 0::2, :],
                    zeros_f[HD:P, 0:1, None].to_broadcast([HD, 2, S]),
                )
                nc.vector.tensor_copy(
                    qTr[0:HD, 1::2, :],
                    zeros_f[0:HD, 0:1, None].to_broadcast([HD, 2, S]),
                )

                with (
                    tc.tile_pool(name="ps_qk", bufs=1, space="PSUM") as ps_qk,
                    tc.tile_pool(name="ps_v", bufs=4, space="PSUM") as ps_v,
                ):
                    for sl in range(N_QC):
                        ssl = slice(sl * QC, (sl + 1) * QC)
                        for name, mt in (
                            ("q", 0), ("q", 1), ("k", 0), ("k", 1),
                        ):
                            wr = wts[name]
                            ps = ps_qk.tile(
                                [P, QC], f32,
                                tag=f"pg{name}{mt}", name=f"pg{name}{mt}",
                            )
                            for ko in range(KO):
                                nc.tensor.matmul(
                                    ps[:],
                                    wr[:, ko, mt * P : (mt + 1) * P],
                                    xTr[:, ko, ssl],
                                    start=(ko == 0),
                                    stop=(ko == KO - 1),
                                )
                            if name == "k":
                                nc.vector.tensor_copy(
                                    kTr[:, mt, ssl], ps[:]
                                )
                            else:
                                nc.vector.tensor_copy(
                                    qTr[0:HD, 2 * mt, ssl], ps[0:HD, :]
                                )
                                nc.vector.tensor_copy(
                                    qTr[HD:P, 2 * mt + 1, ssl], ps[HD:P, :]
                                )
                        # v seq tiles of this slab
                        wr = wts["v"]
                        for st in range(4 * sl, 4 * sl + 4):
                            ps = ps_v.tile([P, HSL], f32, tag="pv")
                            for ko in range(KO):
                                nc.tensor.matmul(
                                    ps[:],
                                    xTr[:, ko, st * P : (st + 1) * P],
                                    wr[:, ko, :],
                                    start=(ko == 0),
                                    stop=(ko == KO - 1),
                                )
                            nc.scalar.activation(
                                vr[:, st, :, 0:HD],
                                ps[:].rearrange("p (h d) -> p h d", d=HD),
                                mybir.ActivationFunctionType.Copy,
                            )

            # ---- phase 2: attention per (query-pair, head)
            with (
                tc.tile_pool(name="expr", bufs=4) as expr,
                tc.tile_pool(name="ps_s", bufs=2, space="PSUM") as ps_s,
                tc.tile_pool(name="ps_ot", bufs=2, space="PSUM") as ps_ot,
            ):

                def normalize(h, qc, ps_o):
                    hm, hb = h // 2, (h % 2) * HD
                    sums = small.tile([1, QC], f32, tag="sums", name="sums")
                    nc.vector.tensor_copy(sums[:], ps_o[HD : HD + 1, :])
                    recip = small.tile([1, QC], f32, tag="recip", name="recip")
                    nc.vector.reciprocal_approx_fast(recip[:], sums[:])
                    bcast = small.tile([HD, QC], f32, tag="bcast", name="bcast")
                    nc.gpsimd.partition_broadcast(bcast[:], recip[:])
                    if DEBUG and h == 0 and qc == 0:
                        oc = small.tile(
                            [HD + 1, QC], f32, tag="oc_d", name="oc_d"
                        )
                        nc.vector.tensor_copy(oc[:], ps_o[:])
                        nc.sync.dma_start(dbg["ot_d"][:], oc[:])
                        nc.sync.dma_start(dbg["recip_d"][:], recip[:])
                        nc.sync.dma_start(dbg["bcast_d"][:], bcast[:])
                    nc.vector.tensor_mul(
                        aTr[hb : hb + HD, hm, qc * QC : (qc + 1) * QC],
                        ps_o[0:HD, :],
                        bcast[:],
                    )

                def o_group(h, okt, segs, er_g, ps_ots):
                    for qc, c0, o0, w in reversed(segs):
                        nc.tensor.matmul(
                            ps_ots[qc][:, o0:QC],
                            vr[:, okt, h, :],
                            er_g[:, c0 : c0 + w],
                            start=(okt == 0),
                            stop=(okt == 4 * qc + 3),
                        )
                        if okt == 4 * qc + 3:
                            normalize(h, qc, ps_ots[qc])

                for pr in range(2):
                    qcs = (2 * pr, 2 * pr + 1)
                    for h in range(NH_CORE):
                        hm = h // 2
                        ps_ots = {
                            qc: ps_ot.tile(
                                [HD + 1, QC], f32,
                                tag=f"ot{qc % 2}", name="ps_ot",
                            )
                            for qc in qcs
                        }
                        pend = []
                        for kt in range(8 * (pr + 1)):
                            jd = kt // 4
                            off = (kt % 4) * P
                            live = [qc for qc in qcs if qc >= jd]
                            ps_g = ps_s.tile(
                                [P, 2 * QC], f32, tag="ps_s", name="ps_g"
                            )
                            er_g = expr.tile(
                                [P, 2 * QC], f32r, tag="er", name="er_g"
                            )
                            segs = []
                            for qc in live:
                                diag = qc == jd
                                # diagonal chunks keep width >= 256: f32r
                                # matmuls under 256 moving cols run at 1/4
                                o0 = min(off, 2 * P) if diag else 0
                                c0 = (qc - qcs[0]) * QC + o0
                                w = QC - o0
                                nc.tensor.matmul(
                                    ps_g[:, c0 : c0 + w],
                                    kTr[:, hm, kt * P : (kt + 1) * P],
                                    qTr[:, h, qc * QC + o0 : (qc + 1) * QC],
                                    start=True,
                                    stop=not diag,
                                )
                                if diag:
                                    # causal bias: += ident.T @ utri-slice
                                    # (-1e9 above the diagonal; exp -> 0)
                                    nc.tensor.matmul(
                                        ps_g[:, c0 : c0 + 2 * P],
                                        ident_sb[:],
                                        utri_sb[:, o0 - off + P : o0 - off + 3 * P],
                                        start=False,
                                        stop=True,
                                    )
                                segs.append((qc, c0, o0, w))
                            g0 = segs[0][1]
                            g1 = segs[-1][1] + segs[-1][3]
                            nc.scalar.activation(
                                er_g[:, g0:g1],
                                ps_g[:, g0:g1],
                                mybir.ActivationFunctionType.Exp,
                                scale=0.125,
                            )
                            if DEBUG and pr == 0 and h == 0 and kt < 2:
                                nc.sync.dma_start(
                                    dbg["er_d"][:, kt, :], er_g[:]
                                )
                            pend.append((kt, segs, er_g))
                            if len(pend) > 1:
                                okt, osegs, oer = pend.pop(0)
                                o_group(h, okt, osegs, oer, ps_ots)
                        okt, osegs, oer = pend.pop(0)
                        o_group(h, okt, osegs, oer, ps_ots)

            if DEBUG:
                nc.sync.dma_start(dbg["qTr_d"][:], qTr[:])
                nc.sync.dma_start(dbg["kTr_d"][:], kTr[:])
                nc.sync.dma_start(dbg["vr_d"][:], vr[:])
                nc.sync.dma_start(dbg["aTr_d"][:], aTr[:])

            # ---- phase 3: partial o_proj [2048, 1024] -> bf16 out
            with (
                tc.tile_pool(name="ps_o", bufs=2, space="PSUM") as ps_o,
                tc.tile_pool(name="outp", bufs=4) as outp,
            ):
                for st in range(N_KT):
                    ps = ps_o.tile([P, DM], f32, tag="po")
                    for nch in range(2):
                        for kt2 in range(2):
                            nc.tensor.matmul(
                                ps[:, nch * QC : (nch + 1) * QC],
                                aTr[:, kt2, st * P : (st + 1) * P],
                                woTr[:, kt2, nch * QC : (nch + 1) * QC],
                                start=(kt2 == 0),
                                stop=(kt2 == 1),
                            )
                    ot = outp.tile([P, DM], bf16, tag="ot")
                    if st % 2 == 0:
                        nc.vector.tensor_copy(ot[:], ps[:])
                    else:
                        nc.scalar.activation(
                            ot[:], ps[:], mybir.ActivationFunctionType.Copy
                        )
                    (nc.sync if st % 2 == 0 else nc.gpsimd).dma_start(
                        out[st * P : (st + 1) * P, :], ot[:]
                    )

    nc.compile()
    return nc


def _make_utri():
    # utri[p, cc] = -200 where p > cc - 128 (cc in [0, 640)); a slice
    # utri[:, u0-off+128 : u0-off+384] biases query cols [u0, u0+256) of a
    # diagonal key tile. After the 0.125 exp scale a masked logit sits at
    # <= -19 nats (exp <= 6e-9, negligible vs row sums >= 1) while staying
    # inside the ACT exp table's domain (huge negatives misbehave).
    p = np.arange(P)[:, None]
    cc = np.arange(5 * P)[None, :]
    return np.where(p > cc - P, np.float32(-200.0), np.float32(0.0))


def make_in_maps(x, Wq, Wk, Wv, Wo):
    utri = _make_utri()
    ident = np.eye(P, dtype=np.float32)

    def wtile(wT):  # [1024, 256] -> [128, 8, 256] (p, ko, m)
        return np.ascontiguousarray(
            wT.reshape(KO, P, HSL).transpose(1, 0, 2)
        )

    in_maps = []
    for c in range(8):
        bi, g = c // 4, c % 4
        sl = slice(g * HSL, (g + 1) * HSL)
        # xS[sl, p, ko, s] = x[bi][sl*512+s, ko*128+p]
        xs = np.ascontiguousarray(
            x[bi].reshape(N_QC, QC, KO, P).transpose(0, 3, 2, 1)
        )
        in_maps.append(
            {
                "xS": xs,
                "wqT": wtile(Wq[sl, :].T),
                "wkT": wtile(Wk[sl, :].T),
                "wvT": wtile(Wv[sl, :].T),
                "woT": np.ascontiguousarray(
                    Wo[:, sl].T.reshape(2, P, DM).transpose(1, 0, 2)
                ),
                "utri": utri,
                "ident": ident,
            }
        )
    return in_maps


def kernel(x, Wq, Wk, Wv, Wo):
    x = np.asarray(x, dtype=np.float32)
    Wq = np.asarray(Wq, dtype=np.float32)
    Wk = np.asarray(Wk, dtype=np.float32)
    Wv = np.asarray(Wv, dtype=np.float32)
    Wo = np.asarray(Wo, dtype=np.float32)
    b, s, dm = x.shape
    assert (b, s, dm) == (2, S, DM), (b, s, dm)

    if "nc" not in _CACHED:
        _CACHED["nc"] = build_program()
    nc = _CACHED["nc"]

    in_maps = make_in_maps(x, Wq, Wk, Wv, Wo)
    res = run_bass_kernel_spmd(nc, in_maps, core_ids=list(range(8)))

    out = np.zeros((2, S, DM), dtype=np.float32)
    for c in range(8):
        out[c // 4] += np.asarray(res.results[c]["out"]).astype(np.float32)
    return out


# revision 27
# speedup vs baseline: 1.1407x; 1.1407x over previous
"""Multi-head self-attention (causal) Trainium2 kernel, 8-way sharded.

Sharding: core c handles batch b = c//4 and head group g = c%4 (4 of 16
heads). Each core computes q/k/v projections for its head slice, causal
softmax attention, and a partial o_proj ([2048, 1024] bf16); the host
sums the 4 partials per batch in f32.

Layouts (per core):
  xT    [1024, 2048]  x[b].T            (d_model on partitions)
  wqT   [1024,  256]  Wq[g*256:(g+1)*256, :].T      (same for wk/wv)
  woT   [ 256, 1024]  Wo[:, g*256:(g+1)*256].T
  utri  [ 128,  640]  -1e9 where p > cc-128 (causal bias, shifted views)
  ident [ 128,  128]  identity (stationary for the causal-bias matmul)

All DRAM params are declared f32r (same bits as f32) and DMA'd straight
into f32r SBUF tiles — no rounding casts. q/k projections run ko-outer
(4 query-chunk PSUM groups live per weight half) so matmuls start as
soon as the first x chunk lands. kT/qT are stored head-major stacked
two heads per partition column; S matmuls contract over K=64 partition
slices so no zero padding is needed.

Attention per (query-pair pr, head): key tiles stream, each kt producing
one [128, <=1024] PSUM group. Diagonal sub-blocks get the causal mask as
an extra accumulated matmul (ident.T @ utri-slice = -1e9 above the
diagonal) so the single wide exp (ACT) yields exact zeros there and no
post-exp mask is needed. Diagonal chunk widths are kept >= 256 (f32r
matmuls below 256 moving columns run at 1/4 rate). O matmuls are issued
one kt-group late; V carries an appended ones column so PSUM partition
64 accumulates the softmax sums; normalization is a custom-DVE
reciprocal_approx_fast + GpSimd partition_broadcast + DVE multiply.
"""

import numpy as np

import concourse.bass as bass
import concourse.mybir as mybir
import concourse.tile as tile
from concourse import bacc
from concourse.bass_utils import run_bass_kernel_spmd

P = 128
S = 2048  # sequence length
DM = 1024  # d_model
HD = 64  # head dim
NH_CORE = 4  # heads per core
HSL = NH_CORE * HD  # head slice width = 256
QC = 512  # query chunk
N_QC = S // QC  # 4
N_KT = S // P  # 16 key tiles
KO = DM // P  # 8 k-tiles over d_model

f32 = mybir.dt.float32
f32r = mybir.dt.float32r
bf16 = mybir.dt.bfloat16

_CACHED = {}
DEBUG = False


def build_program():
    nc = bacc.Bacc("TRN2", target_bir_lowering=False, debug=False)
    # all inputs host-prearranged into SBUF tile layouts so every DMA line
    # is one long contiguous read per partition (no strided descriptors)
    xS = nc.declare_dram_parameter("xS", [N_QC, P, KO, QC], f32r, isOutput=False)
    wqT = nc.declare_dram_parameter("wqT", [P, KO, HSL], f32r, isOutput=False)
    wkT = nc.declare_dram_parameter("wkT", [P, KO, HSL], f32r, isOutput=False)
    wvT = nc.declare_dram_parameter("wvT", [P, KO, HSL], f32r, isOutput=False)
    woT = nc.declare_dram_parameter("woT", [P, 2, DM], f32r, isOutput=False)
    utri = nc.declare_dram_parameter("utri", [P, 5 * P], f32r, isOutput=False)
    ident = nc.declare_dram_parameter("ident", [P, P], f32r, isOutput=False)
    out = nc.declare_dram_parameter("out", [S, DM], bf16, isOutput=True)
    if DEBUG:
        dbg = {
            "qTr_d": nc.declare_dram_parameter(
                "qTr_d", [P, 2, S], f32r, isOutput=True
            ),
            "kTr_d": nc.declare_dram_parameter(
                "kTr_d", [P, 2, S], f32r, isOutput=True
            ),
            "vr_d": nc.declare_dram_parameter(
                "vr_d", [P, N_KT, NH_CORE, HD + 1], f32r, isOutput=True
            ),
            "aTr_d": nc.declare_dram_parameter(
                "aTr_d", [P, 2, S], f32r, isOutput=True
            ),
            "er_d": nc.declare_dram_parameter(
                "er_d", [P, 2, 2 * QC], f32r, isOutput=True
            ),
            "ot_d": nc.declare_dram_parameter(
                "ot_d", [HD + 1, QC], f32, isOutput=True
            ),
            "recip_d": nc.declare_dram_parameter(
                "recip_d", [1, QC], f32, isOutput=True
            ),
            "bcast_d": nc.declare_dram_parameter(
                "bcast_d", [HD, QC], f32, isOutput=True
            ),
        }

    with tile.TileContext(nc) as tc:
        with (
            tc.tile_pool(name="persist", bufs=1) as persist,
            tc.tile_pool(name="small", bufs=2) as small,
        ):
            # ---- persistent tiles
            qTr = persist.tile([P, NH_CORE, S], f32r, tag="qTr")
            kTr = persist.tile([P, 2, S], f32r, tag="kTr")
            vr = persist.tile([P, N_KT, NH_CORE, HD + 1], f32r, tag="vr")
            woTr = persist.tile([P, 2, DM], f32r, tag="woTr")
            aTr = persist.tile([P, 2, S], f32r, tag="aTr")
            utri_sb = persist.tile([P, 5 * P], f32r, tag="utri")
            ident_sb = persist.tile([P, P], f32r, tag="ident")

            # ---- phase 0+1: load x/weights, projections.
            # x streams as 4 seq-slabs of 512 cols (all d_model rows); each
            # slab feeds all four q/k projection halves plus its 4 v seq
            # tiles, so PE consumption (~12us) outpaces slab DMA (~6us) and
            # the tensor engine never idles (keeps the HAM clock at 2.4GHz).
            with tc.tile_pool(name="xw", bufs=1) as xw:
                # slab-major so each slab DMA writes one contiguous
                # 16KB-per-partition block (hardware-dynamic descriptors)
                xTr = xw.tile([P, N_QC, KO, QC], f32r, tag="xTr")
                wts = {}
                for name, dram in (("q", wqT), ("k", wkT), ("v", wvT)):
                    wts[name] = xw.tile(
                        [P, KO, HSL], f32r, tag=f"w{name}r", name=f"w{name}r"
                    )
                # critical path first: wq/wk + slab0 (split across two
                # queues); everything else queued behind
                nc.sync.dma_start(wts["q"][:], wqT[:])
                nc.scalar.dma_start(xTr[:, 0, 0:4, :], xS[0, :, 0:4, :])
                nc.sync.dma_start(xTr[:, 0, 4:8, :], xS[0, :, 4:8, :])
                nc.gpsimd.dma_start(xTr[:, 1], xS[1])
                nc.sync.dma_start(wts["k"][:], wkT[:])
                nc.scalar.dma_start(xTr[:, 2], xS[2])
                nc.gpsimd.dma_start(xTr[:, 3], xS[3])
                nc.sync.dma_start(wts["v"][:], wvT[:])
                nc.sync.dma_start(woTr[:], woT[:])
                nc.sync.dma_start(utri_sb[:], utri[:])
                nc.sync.dma_start(ident_sb[:], ident[:])
                nc.vector.memset(
                    vr[:, :, :, HD]
                    .rearrange("p a b -> p (a b)")
                    .bitcast(mybir.dt.float32),
                    1.0,
                )
                # zero the pad halves of qTr: even heads live on partitions
                # 0:64, odd heads on 64:128 (the other half multiplies the
                # co-resident head's k rows, so it must be zero)
                zeros_f = persist.tile([P, 1], f32, tag="zeros")
                nc.vector.memset(zeros_f[:], 0.0)
                nc.vector.tensor_copy(
                    qTr[HD:P, 0::2, :],
                    zeros_f[HD:P, 0:1, None].to_broadcast([HD, 2, S]),
                )
                nc.vector.tensor_copy(
                    qTr[0:HD, 1::2, :],
                    zeros_f[0:HD, 0:1, None].to_broadcast([HD, 2, S]),
                )

                with (
                    tc.tile_pool(name="ps_qk", bufs=1, space="PSUM") as ps_qk,
                    tc.tile_pool(name="ps_v", bufs=4, space="PSUM") as ps_v,
                ):
                    for sl in range(N_QC):
                        ssl = slice(sl * QC, (sl + 1) * QC)
                        for name, mt in (
                            ("q", 0), ("q", 1), ("k", 0), ("k", 1),
                        ):
                            wr = wts[name]
                            ps = ps_qk.tile(
                                [P, QC], f32,
                                tag=f"pg{name}{mt}", name=f"pg{name}{mt}",
                            )
                            for ko in range(KO):
                                nc.tensor.matmul(
                                    ps[:],
                                    wr[:, ko, mt * P : (mt + 1) * P],
                                    xTr[:, sl, ko, :],
                                    start=(ko == 0),
                                    stop=(ko == KO - 1),
                                )
                            if name == "k":
                                nc.vector.tensor_copy(
                                    kTr[:, mt, ssl], ps[:]
                                )
                            else:
                                nc.vector.tensor_copy(
                                    qTr[0:HD, 2 * mt, ssl], ps[0:HD, :]
                                )
                                nc.vector.tensor_copy(
                                    qTr[HD:P, 2 * mt + 1, ssl], ps[HD:P, :]
                                )
                        # v seq tiles of this slab
                        wr = wts["v"]
                        for st4 in range(4):
                            st = 4 * sl + st4
                            ps = ps_v.tile([P, HSL], f32, tag="pv")
                            for ko in range(KO):
                                nc.tensor.matmul(
                                    ps[:],
                                    xTr[:, sl, ko, st4 * P : (st4 + 1) * P],
                                    wr[:, ko, :],
                                    start=(ko == 0),
                                    stop=(ko == KO - 1),
                                )
                            nc.scalar.activation(
                                vr[:, st, :, 0:HD],
                                ps[:].rearrange("p (h d) -> p h d", d=HD),
                                mybir.ActivationFunctionType.Copy,
                            )

            # ---- phase 2: attention per (query-pair, head)
            with (
                tc.tile_pool(name="expr", bufs=4) as expr,
                tc.tile_pool(name="ps_s", bufs=2, space="PSUM") as ps_s,
                tc.tile_pool(name="ps_ot", bufs=2, space="PSUM") as ps_ot,
            ):

                def normalize(h, qc, ps_o):
                    hm, hb = h // 2, (h % 2) * HD
                    sums = small.tile([1, QC], f32, tag="sums", name="sums")
                    nc.vector.tensor_copy(sums[:], ps_o[HD : HD + 1, :])
                    recip = small.tile([1, QC], f32, tag="recip", name="recip")
                    nc.vector.reciprocal_approx_fast(recip[:], sums[:])
                    bcast = small.tile([HD, QC], f32, tag="bcast", name="bcast")
                    nc.gpsimd.partition_broadcast(bcast[:], recip[:])
                    if DEBUG and h == 0 and qc == 0:
                        oc = small.tile(
                            [HD + 1, QC], f32, tag="oc_d", name="oc_d"
                        )
                        nc.vector.tensor_copy(oc[:], ps_o[:])
                        nc.sync.dma_start(dbg["ot_d"][:], oc[:])
                        nc.sync.dma_start(dbg["recip_d"][:], recip[:])
                        nc.sync.dma_start(dbg["bcast_d"][:], bcast[:])
                    nc.vector.tensor_mul(
                        aTr[hb : hb + HD, hm, qc * QC : (qc + 1) * QC],
                        ps_o[0:HD, :],
                        bcast[:],
                    )

                def o_group(h, okt, segs, er_g, ps_ots):
                    for qc, c0, o0, w in reversed(segs):
                        nc.tensor.matmul(
                            ps_ots[qc][:, o0:QC],
                            vr[:, okt, h, :],
                            er_g[:, c0 : c0 + w],
                            start=(okt == 0),
                            stop=(okt == 4 * qc + 3),
                        )
                        if okt == 4 * qc + 3:
                            normalize(h, qc, ps_ots[qc])

                for pr in range(2):
                    qcs = (2 * pr, 2 * pr + 1)
                    for h in range(NH_CORE):
                        hm = h // 2
                        ps_ots = {
                            qc: ps_ot.tile(
                                [HD + 1, QC], f32,
                                tag=f"ot{qc % 2}", name="ps_ot",
                            )
                            for qc in qcs
                        }
                        pend = []
                        for kt in range(8 * (pr + 1)):
                            jd = kt // 4
                            off = (kt % 4) * P
                            live = [qc for qc in qcs if qc >= jd]
                            ps_g = ps_s.tile(
                                [P, 2 * QC], f32, tag="ps_s", name="ps_g"
                            )
                            er_g = expr.tile(
                                [P, 2 * QC], f32r, tag="er", name="er_g"
                            )
                            segs = []
                            for qc in live:
                                diag = qc == jd
                                # diagonal chunks keep width >= 256: f32r
                                # matmuls under 256 moving cols run at 1/4
                                o0 = min(off, 2 * P) if diag else 0
                                c0 = (qc - qcs[0]) * QC + o0
                                w = QC - o0
                                nc.tensor.matmul(
                                    ps_g[:, c0 : c0 + w],
                                    kTr[:, hm, kt * P : (kt + 1) * P],
                                    qTr[:, h, qc * QC + o0 : (qc + 1) * QC],
                                    start=True,
                                    stop=not diag,
                                )
                                if diag:
                                    # causal bias: += ident.T @ utri-slice
                                    # (-1e9 above the diagonal; exp -> 0)
                                    nc.tensor.matmul(
                                        ps_g[:, c0 : c0 + 2 * P],
                                        ident_sb[:],
                                        utri_sb[:, o0 - off + P : o0 - off + 3 * P],
                                        start=False,
                                        stop=True,
                                    )
                                segs.append((qc, c0, o0, w))
                            g0 = segs[0][1]
                            g1 = segs[-1][1] + segs[-1][3]
                            nc.scalar.activation(
                                er_g[:, g0:g1],
                                ps_g[:, g0:g1],
                                mybir.ActivationFunctionType.Exp,
                                scale=0.125,
                            )
                            if DEBUG and pr == 0 and h == 0 and kt < 2:
                                nc.sync.dma_start(
                                    dbg["er_d"][:, kt, :], er_g[:]
                                )
                            pend.append((kt, segs, er_g))
                            if len(pend) > 1:
                                okt, osegs, oer = pend.pop(0)
                                o_group(h, okt, osegs, oer, ps_ots)
                        okt, osegs, oer = pend.pop(0)
                        o_group(h, okt, osegs, oer, ps_ots)

            if DEBUG:
                nc.sync.dma_start(dbg["qTr_d"][:], qTr[:])
                nc.sync.dma_start(dbg["kTr_d"][:], kTr[:])
                nc.sync.dma_start(dbg["vr_d"][:], vr[:])
                nc.sync.dma_start(dbg["aTr_d"][:], aTr[:])

            # ---- phase 3: partial o_proj [2048, 1024] -> bf16 out
            with (
                tc.tile_pool(name="ps_o", bufs=2, space="PSUM") as ps_o,
                tc.tile_pool(name="outp", bufs=4) as outp,
            ):
                for st in range(N_KT):
                    ps = ps_o.tile([P, DM], f32, tag="po")
                    for nch in range(2):
                        for kt2 in range(2):
                            nc.tensor.matmul(
                                ps[:, nch * QC : (nch + 1) * QC],
                                aTr[:, kt2, st * P : (st + 1) * P],
                                woTr[:, kt2, nch * QC : (nch + 1) * QC],
                                start=(kt2 == 0),
                                stop=(kt2 == 1),
                            )
                    ot = outp.tile([P, DM], bf16, tag="ot")
                    if st % 2 == 0:
                        nc.vector.tensor_copy(ot[:], ps[:])
                    else:
                        nc.scalar.activation(
                            ot[:], ps[:], mybir.ActivationFunctionType.Copy
                        )
                    (nc.sync if st % 2 == 0 else nc.gpsimd).dma_start(
                        out[st * P : (st + 1) * P, :], ot[:]
                    )

    nc.compile()
    return nc


def _make_utri():
    # utri[p, cc] = -200 where p > cc - 128 (cc in [0, 640)); a slice
    # utri[:, u0-off+128 : u0-off+384] biases query cols [u0, u0+256) of a
    # diagonal key tile. After the 0.125 exp scale a masked logit sits at
    # <= -19 nats (exp <= 6e-9, negligible vs row sums >= 1) while staying
    # inside the ACT exp table's domain (huge negatives misbehave).
    p = np.arange(P)[:, None]
    cc = np.arange(5 * P)[None, :]
    return np.where(p > cc - P, np.float32(-200.0), np.float32(0.0))


def make_in_maps(x, Wq, Wk, Wv, Wo):
    utri = _make_utri()
    ident = np.eye(P, dtype=np.float32)

    def wtile(wT):  # [1024, 256] -> [128, 8, 256] (p, ko, m)
        return np.ascontiguousarray(
            wT.reshape(KO, P, HSL).transpose(1, 0, 2)
        )

    in_maps = []
    for c in range(8):
        bi, g = c // 4, c % 4
        sl = slice(g * HSL, (g + 1) * HSL)
        # xS[sl, p, ko, s] = x[bi][sl*512+s, ko*128+p]
        xs = np.ascontiguousarray(
            x[bi].reshape(N_QC, QC, KO, P).transpose(0, 3, 2, 1)
        )
        in_maps.append(
            {
                "xS": xs,
                "wqT": wtile(Wq[sl, :].T),
                "wkT": wtile(Wk[sl, :].T),
                "wvT": wtile(Wv[sl, :].T),
                "woT": np.ascontiguousarray(
                    Wo[:, sl].T.reshape(2, P, DM).transpose(1, 0, 2)
                ),
                "utri": utri,
                "ident": ident,
            }
        )
    return in_maps


def kernel(x, Wq, Wk, Wv, Wo):
    x = np.asarray(x, dtype=np.float32)
    Wq = np.asarray(Wq, dtype=np.float32)
    Wk = np.asarray(Wk, dtype=np.float32)
    Wv = np.asarray(Wv, dtype=np.float32)
    Wo = np.asarray(Wo, dtype=np.float32)
    b, s, dm = x.shape
    assert (b, s, dm) == (2, S, DM), (b, s, dm)

    if "nc" not in _CACHED:
        _CACHED["nc"] = build_program()
    nc = _CACHED["nc"]

    in_maps = make_in_maps(x, Wq, Wk, Wv, Wo)
    res = run_bass_kernel_spmd(nc, in_maps, core_ids=list(range(8)))

    out = np.zeros((2, S, DM), dtype=np.float32)
    for c in range(8):
        out[c // 4] += np.asarray(res.results[c]["out"]).astype(np.float32)
    return out


# revision 31
# speedup vs baseline: 1.2913x; 1.1320x over previous
"""Multi-head self-attention (causal) Trainium2 kernel, 8-way sharded.

Sharding: core c handles batch b = c//4 and head group g = c%4 (4 of 16
heads). Each core computes q/k/v projections for its head slice, causal
softmax attention, and a partial o_proj ([2048, 1024] bf16); the host
sums the 4 partials per batch in f32.

Layouts (per core):
  xT    [1024, 2048]  x[b].T            (d_model on partitions)
  wqT   [1024,  256]  Wq[g*256:(g+1)*256, :].T      (same for wk/wv)
  woT   [ 256, 1024]  Wo[:, g*256:(g+1)*256].T
  utri  [ 128,  640]  -1e9 where p > cc-128 (causal bias, shifted views)
  ident [ 128,  128]  identity (stationary for the causal-bias matmul)

All DRAM params are declared f32r (same bits as f32) and DMA'd straight
into f32r SBUF tiles — no rounding casts. q/k projections run ko-outer
(4 query-chunk PSUM groups live per weight half) so matmuls start as
soon as the first x chunk lands. kT/qT are stored head-major stacked
two heads per partition column; S matmuls contract over K=64 partition
slices so no zero padding is needed.

Attention per (query-pair pr, head): key tiles stream, each kt producing
one [128, <=1024] PSUM group. Diagonal sub-blocks get the causal mask as
an extra accumulated matmul (ident.T @ utri-slice = -1e9 above the
diagonal) so the single wide exp (ACT) yields exact zeros there and no
post-exp mask is needed. Diagonal chunk widths are kept >= 256 (f32r
matmuls below 256 moving columns run at 1/4 rate). O matmuls are issued
one kt-group late; V carries an appended ones column so PSUM partition
64 accumulates the softmax sums; normalization is a custom-DVE
reciprocal_approx_fast + GpSimd partition_broadcast + DVE multiply.
"""

import ml_dtypes
import numpy as np

import concourse.bass as bass
import concourse.mybir as mybir
import concourse.tile as tile
from concourse import bacc
from concourse.bass_utils import run_bass_kernel_spmd

P = 128
S = 2048  # sequence length
DM = 1024  # d_model
HD = 64  # head dim
NH_CORE = 4  # heads per core
HSL = NH_CORE * HD  # head slice width = 256
QC = 512  # query chunk
N_QC = S // QC  # 4
N_KT = S // P  # 16 key tiles
KO = DM // P  # 8 k-tiles over d_model

f32 = mybir.dt.float32
f32r = mybir.dt.float32r
bf16 = mybir.dt.bfloat16

_CACHED = {}
DEBUG = False


def build_program():
    nc = bacc.Bacc("TRN2", target_bir_lowering=False, debug=False)
    # all inputs host-prearranged into SBUF tile layouts so every DMA line
    # is one long contiguous read per partition (no strided descriptors)
    xS = nc.declare_dram_parameter("xS", [N_QC, P, KO, QC], bf16, isOutput=False)
    wqT = nc.declare_dram_parameter("wqT", [P, KO, HSL], bf16, isOutput=False)
    wkT = nc.declare_dram_parameter("wkT", [P, KO, HSL], bf16, isOutput=False)
    wvT = nc.declare_dram_parameter("wvT", [P, KO, HSL], bf16, isOutput=False)
    woT = nc.declare_dram_parameter("woT", [P, 2, DM], bf16, isOutput=False)
    utri = nc.declare_dram_parameter("utri", [P, 5 * P], f32r, isOutput=False)
    ident = nc.declare_dram_parameter("ident", [P, P], f32r, isOutput=False)
    out = nc.declare_dram_parameter("out", [S, DM], bf16, isOutput=True)
    if DEBUG:
        dbg = {
            "qTr_d": nc.declare_dram_parameter(
                "qTr_d", [P, 2, S], f32r, isOutput=True
            ),
            "kTr_d": nc.declare_dram_parameter(
                "kTr_d", [P, 2, S], f32r, isOutput=True
            ),
            "vr_d": nc.declare_dram_parameter(
                "vr_d", [P, N_KT, NH_CORE, HD + 1], f32r, isOutput=True
            ),
            "aTr_d": nc.declare_dram_parameter(
                "aTr_d", [P, 2, S], f32r, isOutput=True
            ),
            "er_d": nc.declare_dram_parameter(
                "er_d", [P, 2, 2 * QC], f32r, isOutput=True
            ),
            "ot_d": nc.declare_dram_parameter(
                "ot_d", [HD + 1, QC], f32, isOutput=True
            ),
            "recip_d": nc.declare_dram_parameter(
                "recip_d", [1, QC], f32, isOutput=True
            ),
            "bcast_d": nc.declare_dram_parameter(
                "bcast_d", [HD, QC], f32, isOutput=True
            ),
        }

    with tile.TileContext(nc) as tc:
        with (
            tc.tile_pool(name="persist", bufs=1) as persist,
            tc.tile_pool(name="small", bufs=2) as small,
        ):
            # ---- persistent tiles
            qTr = persist.tile([P, NH_CORE, S], f32r, tag="qTr")
            kTr = persist.tile([P, 2, S], f32r, tag="kTr")
            vr = persist.tile([P, N_KT, NH_CORE, HD + 1], f32r, tag="vr")
            woTr = persist.tile([P, 2, DM], bf16, tag="woTr")
            aTr = persist.tile([P, 2, S], bf16, tag="aTr")
            utri_sb = persist.tile([P, 5 * P], f32r, tag="utri")
            ident_sb = persist.tile([P, P], f32r, tag="ident")

            # ---- phase 0+1: load x/weights, projections.
            # x streams as 4 seq-slabs of 512 cols (all d_model rows); each
            # slab feeds all four q/k projection halves plus its 4 v seq
            # tiles, so PE consumption (~12us) outpaces slab DMA (~6us) and
            # the tensor engine never idles (keeps the HAM clock at 2.4GHz).
            with tc.tile_pool(name="xw", bufs=1) as xw:
                # slab-major so each slab DMA writes one contiguous
                # 16KB-per-partition block (hardware-dynamic descriptors)
                xTr = xw.tile([P, N_QC, KO, QC], bf16, tag="xTr")
                wts = {}
                for name, dram in (("q", wqT), ("k", wkT), ("v", wvT)):
                    wts[name] = xw.tile(
                        [P, KO, HSL], bf16, tag=f"w{name}r", name=f"w{name}r"
                    )
                # critical path first: wq/wk + slab0 (split across two
                # queues); everything else queued behind
                nc.sync.dma_start(wts["q"][:], wqT[:])
                nc.scalar.dma_start(xTr[:, 0, 0:4, :], xS[0, :, 0:4, :])
                nc.sync.dma_start(xTr[:, 0, 4:8, :], xS[0, :, 4:8, :])
                nc.gpsimd.dma_start(xTr[:, 1], xS[1])
                nc.sync.dma_start(wts["k"][:], wkT[:])
                nc.scalar.dma_start(xTr[:, 2], xS[2])
                nc.gpsimd.dma_start(xTr[:, 3], xS[3])
                nc.sync.dma_start(wts["v"][:], wvT[:])
                nc.sync.dma_start(woTr[:], woT[:])
                nc.sync.dma_start(utri_sb[:], utri[:])
                nc.sync.dma_start(ident_sb[:], ident[:])
                nc.vector.memset(
                    vr[:, :, :, HD]
                    .rearrange("p a b -> p (a b)")
                    .bitcast(mybir.dt.float32),
                    1.0,
                )
                # zero the pad halves of qTr: even heads live on partitions
                # 0:64, odd heads on 64:128 (the other half multiplies the
                # co-resident head's k rows, so it must be zero)
                zeros_f = persist.tile([P, 1], f32, tag="zeros")
                nc.vector.memset(zeros_f[:], 0.0)
                nc.vector.tensor_copy(
                    qTr[HD:P, 0::2, :],
                    zeros_f[HD:P, 0:1, None].to_broadcast([HD, 2, S]),
                )
                nc.vector.tensor_copy(
                    qTr[0:HD, 1::2, :],
                    zeros_f[0:HD, 0:1, None].to_broadcast([HD, 2, S]),
                )

                with (
                    tc.tile_pool(name="ps_qk", bufs=1, space="PSUM") as ps_qk,
                    tc.tile_pool(name="ps_v", bufs=4, space="PSUM") as ps_v,
                ):
                    for sl in range(N_QC):
                        ssl = slice(sl * QC, (sl + 1) * QC)
                        for name, mt in (
                            ("q", 0), ("q", 1), ("k", 0), ("k", 1),
                        ):
                            wr = wts[name]
                            ps = ps_qk.tile(
                                [P, QC], f32,
                                tag=f"pg{name}{mt}", name=f"pg{name}{mt}",
                            )
                            for ko in range(KO):
                                nc.tensor.matmul(
                                    ps[:],
                                    wr[:, ko, mt * P : (mt + 1) * P],
                                    xTr[:, sl, ko, :],
                                    start=(ko == 0),
                                    stop=(ko == KO - 1),
                                )
                            if name == "k":
                                nc.vector.tensor_copy(
                                    kTr[:, mt, ssl], ps[:]
                                )
                            else:
                                nc.vector.tensor_copy(
                                    qTr[0:HD, 2 * mt, ssl], ps[0:HD, :]
                                )
                                nc.vector.tensor_copy(
                                    qTr[HD:P, 2 * mt + 1, ssl], ps[HD:P, :]
                                )
                        # v seq tiles of this slab
                        wr = wts["v"]
                        for st4 in range(4):
                            st = 4 * sl + st4
                            ps = ps_v.tile([P, HSL], f32, tag="pv")
                            for ko in range(KO):
                                nc.tensor.matmul(
                                    ps[:],
                                    xTr[:, sl, ko, st4 * P : (st4 + 1) * P],
                                    wr[:, ko, :],
                                    start=(ko == 0),
                                    stop=(ko == KO - 1),
                                )
                            nc.scalar.activation(
                                vr[:, st, :, 0:HD],
                                ps[:].rearrange("p (h d) -> p h d", d=HD),
                                mybir.ActivationFunctionType.Copy,
                            )

            # ---- phase 2: attention per (query-pair, head)
            with (
                tc.tile_pool(name="expr", bufs=4) as expr,
                tc.tile_pool(name="ps_s", bufs=2, space="PSUM") as ps_s,
                tc.tile_pool(name="ps_ot", bufs=2, space="PSUM") as ps_ot,
            ):

                def normalize(h, qc, ps_o):
                    hm, hb = h // 2, (h % 2) * HD
                    sums = small.tile([1, QC], f32, tag="sums", name="sums")
                    nc.vector.tensor_copy(sums[:], ps_o[HD : HD + 1, :])
                    recip = small.tile([1, QC], f32, tag="recip", name="recip")
                    nc.vector.reciprocal_approx_fast(recip[:], sums[:])
                    bcast = small.tile([HD, QC], f32, tag="bcast", name="bcast")
                    nc.gpsimd.partition_broadcast(bcast[:], recip[:])
                    if DEBUG and h == 0 and qc == 0:
                        oc = small.tile(
                            [HD + 1, QC], f32, tag="oc_d", name="oc_d"
                        )
                        nc.vector.tensor_copy(oc[:], ps_o[:])
                        nc.sync.dma_start(dbg["ot_d"][:], oc[:])
                        nc.sync.dma_start(dbg["recip_d"][:], recip[:])
                        nc.sync.dma_start(dbg["bcast_d"][:], bcast[:])
                    nc.vector.tensor_mul(
                        aTr[hb : hb + HD, hm, qc * QC : (qc + 1) * QC],
                        ps_o[0:HD, :],
                        bcast[:],
                    )

                def o_group(h, okt, segs, er_g, ps_ots):
                    for qc, c0, o0, w in reversed(segs):
                        nc.tensor.matmul(
                            ps_ots[qc][:, o0:QC],
                            vr[:, okt, h, :],
                            er_g[:, c0 : c0 + w],
                            start=(okt == 0),
                            stop=(okt == 4 * qc + 3),
                        )
                        if okt == 4 * qc + 3:
                            normalize(h, qc, ps_ots[qc])

                for pr in range(2):
                    qcs = (2 * pr, 2 * pr + 1)
                    for h in range(NH_CORE):
                        hm = h // 2
                        ps_ots = {
                            qc: ps_ot.tile(
                                [HD + 1, QC], f32,
                                tag=f"ot{qc % 2}", name="ps_ot",
                            )
                            for qc in qcs
                        }
                        pend = []
                        for kt in range(8 * (pr + 1)):
                            jd = kt // 4
                            off = (kt % 4) * P
                            live = [qc for qc in qcs if qc >= jd]
                            ps_g = ps_s.tile(
                                [P, 2 * QC], f32, tag="ps_s", name="ps_g"
                            )
                            er_g = expr.tile(
                                [P, 2 * QC], f32r, tag="er", name="er_g"
                            )
                            segs = []
                            for qc in live:
                                diag = qc == jd
                                # diagonal chunks keep width >= 256: f32r
                                # matmuls under 256 moving cols run at 1/4
                                o0 = min(off, 2 * P) if diag else 0
                                c0 = (qc - qcs[0]) * QC + o0
                                w = QC - o0
                                nc.tensor.matmul(
                                    ps_g[:, c0 : c0 + w],
                                    kTr[:, hm, kt * P : (kt + 1) * P],
                                    qTr[:, h, qc * QC + o0 : (qc + 1) * QC],
                                    start=True,
                                    stop=not diag,
                                )
                                if diag:
                                    # causal bias: += ident.T @ utri-slice
                                    # (-1e9 above the diagonal; exp -> 0)
                                    nc.tensor.matmul(
                                        ps_g[:, c0 : c0 + 2 * P],
                                        ident_sb[:],
                                        utri_sb[:, o0 - off + P : o0 - off + 3 * P],
                                        start=False,
                                        stop=True,
                                    )
                                segs.append((qc, c0, o0, w))
                            g0 = segs[0][1]
                            g1 = segs[-1][1] + segs[-1][3]
                            nc.scalar.activation(
                                er_g[:, g0:g1],
                                ps_g[:, g0:g1],
                                mybir.ActivationFunctionType.Exp,
                                scale=0.125,
                            )
                            if DEBUG and pr == 0 and h == 0 and kt < 2:
                                nc.sync.dma_start(
                                    dbg["er_d"][:, kt, :], er_g[:]
                                )
                            pend.append((kt, segs, er_g))
                            if len(pend) > 1:
                                okt, osegs, oer = pend.pop(0)
                                o_group(h, okt, osegs, oer, ps_ots)
                        okt, osegs, oer = pend.pop(0)
                        o_group(h, okt, osegs, oer, ps_ots)

            if DEBUG:
                nc.sync.dma_start(dbg["qTr_d"][:], qTr[:])
                nc.sync.dma_start(dbg["kTr_d"][:], kTr[:])
                nc.sync.dma_start(dbg["vr_d"][:], vr[:])
                nc.sync.dma_start(dbg["aTr_d"][:], aTr[:])

            # ---- phase 3: partial o_proj [2048, 1024] -> bf16 out
            with (
                tc.tile_pool(name="ps_o", bufs=2, space="PSUM") as ps_o,
                tc.tile_pool(name="outp", bufs=4) as outp,
            ):
                for st in range(N_KT):
                    ps = ps_o.tile([P, DM], f32, tag="po")
                    for nch in range(2):
                        for kt2 in range(2):
                            nc.tensor.matmul(
                                ps[:, nch * QC : (nch + 1) * QC],
                                aTr[:, kt2, st * P : (st + 1) * P],
                                woTr[:, kt2, nch * QC : (nch + 1) * QC],
                                start=(kt2 == 0),
                                stop=(kt2 == 1),
                            )
                    ot = outp.tile([P, DM], bf16, tag="ot")
                    if st % 2 == 0:
                        nc.vector.tensor_copy(ot[:], ps[:])
                    else:
                        nc.scalar.activation(
                            ot[:], ps[:], mybir.ActivationFunctionType.Copy
                        )
                    [nc.sync, nc.gpsimd, nc.scalar][st % 3].dma_start(
                        out[st * P : (st + 1) * P, :], ot[:]
                    )

    nc.compile()
    return nc


def _make_utri():
    # utri[p, cc] = -200 where p > cc - 128 (cc in [0, 640)); a slice
    # utri[:, u0-off+128 : u0-off+384] biases query cols [u0, u0+256) of a
    # diagonal key tile. After the 0.125 exp scale a masked logit sits at
    # <= -19 nats (exp <= 6e-9, negligible vs row sums >= 1) while staying
    # inside the ACT exp table's domain (huge negatives misbehave).
    p = np.arange(P)[:, None]
    cc = np.arange(5 * P)[None, :]
    return np.where(p > cc - P, np.float32(-200.0), np.float32(0.0))


def make_in_maps(x, Wq, Wk, Wv, Wo):
    utri = _make_utri()
    ident = np.eye(P, dtype=np.float32)

    def wtile(wT):  # [1024, 256] -> [128, 8, 256] (p, ko, m), bf16
        return np.ascontiguousarray(
            wT.reshape(KO, P, HSL).transpose(1, 0, 2)
        ).astype(ml_dtypes.bfloat16)

    in_maps = []
    for c in range(8):
        bi, g = c // 4, c % 4
        sl = slice(g * HSL, (g + 1) * HSL)
        # xS[sl, p, ko, s] = x[bi][sl*512+s, ko*128+p]
        xs = np.ascontiguousarray(
            x[bi].reshape(N_QC, QC, KO, P).transpose(0, 3, 2, 1)
        ).astype(ml_dtypes.bfloat16)
        in_maps.append(
            {
                "xS": xs,
                "wqT": wtile(Wq[sl, :].T),
                "wkT": wtile(Wk[sl, :].T),
                "wvT": wtile(Wv[sl, :].T),
                "woT": np.ascontiguousarray(
                    Wo[:, sl].T.reshape(2, P, DM).transpose(1, 0, 2)
                ).astype(ml_dtypes.bfloat16),
                "utri": utri,
                "ident": ident,
            }
        )
    return in_maps


def kernel(x, Wq, Wk, Wv, Wo):
    x = np.asarray(x, dtype=np.float32)
    Wq = np.asarray(Wq, dtype=np.float32)
    Wk = np.asarray(Wk, dtype=np.float32)
    Wv = np.asarray(Wv, dtype=np.float32)
    Wo = np.asarray(Wo, dtype=np.float32)
    b, s, dm = x.shape
    assert (b, s, dm) == (2, S, DM), (b, s, dm)

    if "nc" not in _CACHED:
        _CACHED["nc"] = build_program()
    nc = _CACHED["nc"]

    in_maps = make_in_maps(x, Wq, Wk, Wv, Wo)
    res = run_bass_kernel_spmd(nc, in_maps, core_ids=list(range(8)))

    out = np.zeros((2, S, DM), dtype=np.float32)
    for c in range(8):
        out[c // 4] += np.asarray(res.results[c]["out"]).astype(np.float32)
    return out
